# revision 17
# baseline (speedup 1.0000x reference)
"""CgsGraphConv (gnn message passing) Trainium2 kernel.

Data-parallel over nodes: each of the 8 cores owns BN/8 = 576 nodes
(9216 edges).  Neighbor indices are global, so every core carries the
full (bf16) node-feature table in its own HBM and gathers neighbor rows
with dma_gather.

Per-core pipeline:
  1. dma_gather centre rows (bit-packed fp32 in a 256B-padded table),
     compute per-edge polar coords + Gaussian mixture edge weights
     (ACT/DVE), normalize, multiply by graph weights.
  2. dma_gather neighbor feature rows (bf16, 2KB each) in 9 chunks.
  3. Aggregation matmuls: per group of 128 edges (8 nodes), lhsT =
     gathered features [128e x 128c], rhs = block-diag masked edge
     weights [128e x 64(node x kernel)] -> psum [128c x 64] = agg^T.
  4. Projection matmuls: contract c in 8 chunks of 128; lhsT = agg^T
     slice [128c x nodes], rhs = conv_w^T slice [128c x 128d] ->
     psum [nodes x 128d]; ReLU -> out rows.
"""

import os
import sys

for _p in ("/opt/trn_rl_repo",):
    if _p not in sys.path and os.path.isdir(_p):
        sys.path.insert(0, _p)

from contextlib import ExitStack

import numpy as np
import ml_dtypes

import concourse.bass as bass
from concourse import bacc
import concourse.mybir as mybir
import concourse.tile as tile
from concourse import bass_utils

BF16 = ml_dtypes.bfloat16

B, N, C = 128, 36, 1024
K_NB = 16
KERNEL = 8
OUT = 1024
DK = OUT // KERNEL          # 128
BN = B * N                  # 4608
NCORES = 8
NODES_PC = BN // NCORES     # 576
E_PC = NODES_PC * K_NB      # 9216
P = 128
NG = E_PC // P              # 72 groups of 128 edges (8 nodes each)
NCALLS = 9                  # feature gather calls, 1024 idx each
GPC = NG // NCALLS          # 8 groups per gather call
CS = C // P                 # 8 contraction chunks
PI = float(np.pi)
HALF_PI = PI / 2.0

LAST_EXEC_NS = None


def _build_program(scalars):
    """Build the SPMD Bass program (identical on all cores)."""
    f32 = mybir.dt.float32
    bf16 = mybir.dt.bfloat16
    i16 = mybir.dt.int16
    u16 = mybir.dt.uint16
    AF = mybir.ActivationFunctionType

    nc = bacc.Bacc("TRN2", target_bir_lowering=False, debug=False, num_devices=NCORES)

    feat = nc.dram_tensor("feat", [BN, C], bf16, kind="ExternalInput").ap()
    idxd = nc.dram_tensor("idx16", [P, E_PC // 16], i16, kind="ExternalInput").ap()
    ctro = nc.dram_tensor("ctro", [P, 2 * NG], f32, kind="ExternalInput").ap()
    ctrn = nc.dram_tensor("ctrn", [P, 2 * NG], f32, kind="ExternalInput").ap()
    gwd = nc.dram_tensor("gw", [P, NG], f32, kind="ExternalInput").ap()
    wtd = nc.dram_tensor("wt", [P, CS * KERNEL * DK], bf16, kind="ExternalInput").ap()
    maskd = nc.dram_tensor("mask", [P, 64], bf16, kind="ExternalInput").ap()
    kpard = nc.dram_tensor("kpar", [P, 4 * KERNEL], f32, kind="ExternalInput").ap()
    outd = nc.dram_tensor("out", [NODES_PC, OUT], f32, kind="ExternalOutput").ap()

    with tile.TileContext(nc) as tc, ExitStack() as ctx:
        constp = ctx.enter_context(tc.tile_pool(name="const", bufs=1))
        wmp = ctx.enter_context(tc.tile_pool(name="wmath", bufs=1))

        idx_sb = constp.tile([P, E_PC // 16], i16)
        nc.sync.dma_start(idx_sb[:], idxd)
        # issue all feature gathers as early as possible (GpSimd is the
        # serial bottleneck; everything else overlaps them)
        fgp = ctx.enter_context(tc.tile_pool(name="fg", bufs=3))
        nidx_reg = nc.gpsimd.to_reg(GPC * P)
        fgs = []
        for j in range(NCALLS):
            fg = fgp.tile([P, GPC * C], bf16, name=f"fg{j}", tag="fg")
            fgv = fg[:].rearrange("p (s e) -> p s e", e=C)
            nc.gpsimd.dma_gather(
                out_ap=fgv,
                in_ap=feat,
                idxs_ap=idx_sb[:, 64 * j : 64 * (j + 1)],
                num_idxs=GPC * P,
                num_idxs_reg=nidx_reg,
                elem_size=C,
            )
            fgs.append(fgv)
        ctr_own = constp.tile([P, 2 * NG], f32)
        nc.sync.dma_start(ctr_own[:], ctro)
        ctr_nbr = constp.tile([P, 2 * NG], f32)
        nc.sync.dma_start(ctr_nbr[:], ctrn)
        gw_sb = constp.tile([P, NG], f32)
        nc.sync.dma_start(gw_sb[:], gwd)
        mask_sb = constp.tile([P, 64], bf16)
        nc.sync.dma_start(mask_sb[:], maskd)
        wt_sb = constp.tile([P, CS * KERNEL * DK], bf16)
        nc.sync.dma_start(wt_sb[:], wtd)

        cj = ctr_nbr[:]  # [P, 2*NG]: (g, (x, y)) of neighbor (host-packed)

        # --- per-edge polar coords ------------------------------------
        dxy = wmp.tile([P, 2 * NG], f32)
        sqt = wmp.tile([P, 2 * NG], f32)
        rho2 = wmp.tile([P, NG], f32)
        rho = wmp.tile([P, NG], f32)
        den = wmp.tile([P, NG], f32)
        rec = wmp.tile([P, NG], f32)
        rat = wmp.tile([P, NG], f32)
        aa = wmp.tile([P, NG], f32)

        nc.vector.tensor_sub(dxy[:], ctr_own[:], cj)
        nc.vector.tensor_mul(sqt[:], dxy[:], dxy[:])
        sq2 = sqt[:].rearrange("p (g two) -> p g two", two=2)
        d2 = dxy[:].rearrange("p (g two) -> p g two", two=2)
        nc.vector.tensor_add(rho2[:], sq2[:, :, 0], sq2[:, :, 1])
        # rho = sqrt(rho2) via exp(0.5*ln(rho2)): keeps ACT inside the
        # natural_log_exp table set (sqrt lives in a different set and
        # would cost an extra ~2.7us table load on the critical path)
        nc.vector.tensor_scalar_max(rho2[:], rho2[:], 1e-30)
        nc.scalar.activation(rho[:], rho2[:], AF.Ln)
        nc.scalar.activation(rho[:], rho[:], AF.Exp, scale=0.5)
        # theta = 2*atan(dx / (rho + dy)); guard rho+dy==0 (self loops)
        nc.vector.tensor_add(den[:], rho[:], d2[:, :, 1])
        nc.vector.tensor_scalar_max(den[:], den[:], 1e-20)
        nc.vector.reciprocal(rec[:], den[:])
        nc.vector.tensor_mul(rat[:], d2[:, :, 0], rec[:])
        # HW Arctan domain is [-pi/2, pi/2]; range-reduce:
        #   atan(t) = sign(t) * ( f*(pi/2) + (1-2f)*atan(min(|t|, 1/|t|)) ),
        #   f = (|t| > 1)
        abt = wmp.tile([P, NG], f32)
        invt = wmp.tile([P, NG], f32)
        mint = wmp.tile([P, NG], f32)
        att = wmp.tile([P, NG], f32)
        fgt = wmp.tile([P, NG], f32)
        qt = wmp.tile([P, NG], f32)
        sgn = wmp.tile([P, NG], f32)
        nc.scalar.activation(abt[:], rat[:], AF.Abs)
        nc.vector.tensor_scalar_max(abt[:], abt[:], 1e-30)
        nc.vector.reciprocal(invt[:], abt[:])
        nc.vector.tensor_tensor(mint[:], abt[:], invt[:], op=mybir.AluOpType.min)
        nc.scalar.activation(att[:], mint[:], AF.Arctan)
        nc.vector.tensor_scalar(
            fgt[:], abt[:], 1.0, None, op0=mybir.AluOpType.is_gt
        )
        # q = 1 - 2f ; aa_abs = q*at + f*pi/2 ; aa = sign(rat) * aa_abs
        # (two-scalar tensor_scalar hits a DVE slow path -- use single ops)
        nc.vector.tensor_scalar_mul(qt[:], fgt[:], -2.0)
        nc.vector.tensor_scalar_add(qt[:], qt[:], 1.0)
        nc.vector.tensor_mul(att[:], att[:], qt[:])
        nc.vector.tensor_scalar_mul(fgt[:], fgt[:], HALF_PI)
        nc.vector.tensor_add(att[:], att[:], fgt[:])
        nc.scalar.activation(sgn[:], rat[:], AF.Sign)
        nc.vector.tensor_mul(aa[:], sgn[:], att[:])  # = theta / 2

        # --- Gaussian mixture weights over the 8 kernels --------------
        # k-batched on [P, NG, KERNEL] tiles with per-kernel parameter
        # rows (kpar input) broadcast along g; per-partition float biases
        # for ACT ops are constant across k.
        _bias_cache = {}

        def bias_ap(val):
            val = float(val)
            if val == 0.0:
                return 0.0  # pre-registered const
            if val not in _bias_cache:
                t = wmp.tile([P, 1], f32, name=f"bias{len(_bias_cache)}")
                nc.vector.memset(t[:], val)
                _bias_cache[val] = t
            return _bias_cache[val][:]

        AL = mybir.AluOpType
        kpar_sb = constp.tile([P, 4 * KERNEL], f32)
        nc.sync.dma_start(kpar_sb[:], kpard)
        kview = kpar_sb[:].rearrange("p (f k) -> p f k", k=KERNEL)

        def kp(i):  # [P, NG, KERNEL] broadcast of param row i
            return kview[:, i : i + 1, :].broadcast_to([P, NG, KERNEL])

        def gb(t):  # [P, NG] tile broadcast along k
            return t[:].unsqueeze(2).broadcast_to([P, NG, KERNEL])

        dkt = wmp.tile([P, NG * KERNEL], f32)
        ukt = wmp.tile([P, NG * KERNEL], f32)
        a1t = wmp.tile([P, NG * KERNEL], f32)
        vvt = wmp.tile([P, NG * KERNEL], f32)
        wall = wmp.tile([P, NG * KERNEL], f32)
        d3 = dkt[:].rearrange("p (g k) -> p g k", k=KERNEL)
        u3 = ukt[:].rearrange("p (g k) -> p g k", k=KERNEL)
        a3 = a1t[:].rearrange("p (g k) -> p g k", k=KERNEL)
        v3 = vvt[:].rearrange("p (g k) -> p g k", k=KERNEL)
        wall3 = wall[:].rearrange("p (g k) -> p g k", k=KERNEL)
        # u = (rho - mr)^2 * (0.5/var_r)
        nc.vector.tensor_tensor(d3, gb(rho), kp(0), op=AL.subtract)
        nc.scalar.activation(ukt[:], dkt[:], AF.Square)
        nc.vector.tensor_tensor(u3, u3, kp(1), op=AL.mult)
        # circular distance via half angle: m/2 = pi/2 - ||aa - mt/2| - pi/2|
        nc.vector.tensor_tensor(d3, gb(aa), kp(2), op=AL.subtract)
        nc.scalar.activation(a1t[:], dkt[:], AF.Abs)
        nc.scalar.activation(a1t[:], a1t[:], AF.Abs, bias=bias_ap(-HALF_PI))
        # v = (pi/2 - a1)^2 * (2/var_t)
        nc.scalar.activation(vvt[:], a1t[:], AF.Square, bias=bias_ap(HALF_PI),
                             scale=-1.0)
        nc.vector.tensor_tensor(v3, v3, kp(3), op=AL.mult)
        nc.vector.tensor_add(ukt[:], ukt[:], vvt[:])
        nc.scalar.activation(wall[:], ukt[:], AF.Exp, scale=-1.0)

        # --- normalize + graph weights + block-diag masks -------------
        wsum = wmp.tile([P, NG], f32)
        wrec = wmp.tile([P, NG], f32)
        gwn = wmp.tile([P, NG], f32)
        nc.vector.tensor_add(wsum[:], wall3[:, :, 0], wall3[:, :, 1])
        for k in range(2, KERNEL):
            nc.vector.tensor_add(wsum[:], wsum[:], wall3[:, :, k])
        nc.vector.reciprocal(wrec[:], wsum[:])
        nc.vector.tensor_mul(gwn[:], gw_sb[:], wrec[:])

        wnorm = wmp.tile([P, NG * KERNEL], bf16)
        nc.vector.tensor_tensor(
            wnorm[:].rearrange("p (g k) -> p g k", k=KERNEL),
            wall3,
            gwn[:].unsqueeze(2).broadcast_to([P, NG, KERNEL]),
            op=mybir.AluOpType.mult,
        )
        # bd[p, g, a, k] = wnorm[p, g, k] * mask[p, a, k]
        bdall = wmp.tile([P, NG * 64], bf16)
        nc.vector.tensor_tensor(
            bdall[:].rearrange("p (g a k) -> p g a k", a=8, k=KERNEL),
            wnorm[:]
            .rearrange("p (g k) -> p g k", k=KERNEL)
            .unsqueeze(2)
            .broadcast_to([P, NG, 8, KERNEL]),
            mask_sb[:]
            .rearrange("p (a k) -> p a k", k=KERNEL)
            .unsqueeze(1)
            .broadcast_to([P, NG, 8, KERNEL]),
            op=mybir.AluOpType.mult,
        )
        bd3 = bdall[:].rearrange("p (g m) -> p g m", m=64)

        # --- aggregation matmuls --------------------------------------
        aggp = ctx.enter_context(tc.tile_pool(name="aggT", bufs=1))
        outp = ctx.enter_context(tc.tile_pool(name="outb", bufs=2))
        aggpsp = ctx.enter_context(tc.tile_pool(name="aps", bufs=3, space="PSUM"))
        projpsp = ctx.enter_context(tc.tile_pool(name="pps", bufs=4, space="PSUM"))

        aggT = aggp.tile([P, CS * NODES_PC * KERNEL], bf16)
        aggT3 = aggT[:].rearrange("p (cs ik) -> p cs ik", cs=CS)
        aggT4 = aggT[:].rearrange(
            "p (cs i k) -> p cs i k", cs=CS, k=KERNEL
        )  # [P, CS, NODES_PC, KERNEL]
        wtv = wt_sb[:].rearrange("p (cs k d) -> p cs k d", cs=CS, k=KERNEL)
        iblocks = [(0, 128), (128, 128), (256, 128), (384, 128), (512, 64)]

        proj_q = [(b, k) for b in range(4) for k in range(KERNEL)]

        outbs = {}

        def emit_proj_slice(ib, k):
            i0, mi = iblocks[ib]
            if ib not in outbs:
                outbs[ib] = outp.tile([P, OUT], mybir.dt.float32,
                                      name=f"outb{ib}", tag="outb")
            outb = outbs[ib]
            pps = projpsp.tile([P, DK], mybir.dt.float32,
                               name=f"pps{ib}_{k}", tag="pps")
            for cs in range(CS):
                nc.tensor.matmul(
                    pps[:mi, :],
                    aggT4[:, cs, i0 : i0 + mi, k],
                    wtv[:, cs, k, :],
                    start=(cs == 0),
                    stop=(cs == CS - 1),
                )
            nc.scalar.activation(
                outb[:mi, k * DK : (k + 1) * DK], pps[:mi, :], AF.Relu
            )
            if k == KERNEL - 1:
                nc.sync.dma_start(outd[i0 : i0 + mi, :], outb[:mi, :])

        for j in range(NCALLS):
            fgv = fgs[j]
            if j == NCALLS - 1:
                # last call: drain every already-eligible projection slice
                # before its aggregation so PE works while the final
                # gather is still in flight
                while proj_q and proj_q[0][0] <= (j - 2) // 2:
                    emit_proj_slice(*proj_q.pop(0))
            for s in range(GPC):
                g = GPC * j + s
                aps = aggpsp.tile([P, CS * 64], mybir.dt.float32,
                                  name=f"aps{g}", tag="aps")
                for cs in range(CS):
                    nc.tensor.matmul(
                        aps[:, cs * 64 : (cs + 1) * 64],
                        fgv[:, s, cs * P : (cs + 1) * P],
                        bd3[:, g, :],
                        start=True,
                        stop=True,
                    )
                if g % 2 == 0:
                    nc.vector.tensor_copy(
                        aggT3[:, :, g * 64 : (g + 1) * 64],
                        aps[:].rearrange("p (cs m) -> p cs m", cs=CS),
                    )
                else:
                    nc.scalar.copy(
                        aggT3[:, :, g * 64 : (g + 1) * 64],
                        aps[:].rearrange("p (cs m) -> p cs m", cs=CS),
                    )
                # interleave one projection slice (8 matmuls) per group,
                # once its i-block's aggregates are fully written:
                # ib b needs gather calls 2b, 2b+1 -> ready from call 2b+2
                if proj_q and proj_q[0][0] <= j // 2 - 1:
                    emit_proj_slice(*proj_q.pop(0))
        while proj_q:
            emit_proj_slice(*proj_q.pop(0))
        for k in range(KERNEL):
            emit_proj_slice(4, k)

    nc.compile()
    return nc


def _host_pack(inputs):
    """Shard + pack host inputs into per-core in_maps and scalars."""
    nf = np.ascontiguousarray(
        np.asarray(inputs["node_feats"], np.float32).reshape(BN, C)
    )
    ctr = np.ascontiguousarray(
        np.asarray(inputs["node_centre"], np.float32).reshape(BN, 2)
    )
    idx = np.asarray(inputs["neighbor_idx"]).reshape(BN, K_NB)
    gwf = np.asarray(inputs["graph_weights"], np.float32).reshape(BN, K_NB)
    mean_rho = np.asarray(inputs["mean_rho"], np.float32).reshape(KERNEL)
    mean_theta = np.asarray(inputs["mean_theta"], np.float32).reshape(KERNEL)
    prec_rho = np.asarray(inputs["precision_rho"], np.float32).reshape(KERNEL)
    prec_theta = np.asarray(inputs["precision_theta"], np.float32).reshape(KERNEL)
    conv_w = np.asarray(inputs["conv_w"], np.float32)

    scalars = {}
    kpar1 = np.concatenate([
        mean_rho,
        0.5 / (1e-14 + prec_rho ** 2),
        mean_theta / 2.0,
        2.0 / (1e-14 + prec_theta ** 2),
    ]).astype(np.float32)  # [32]
    kpar = np.ascontiguousarray(np.tile(kpar1[None, :], (P, 1)))

    feat_tbl = nf.astype(BF16)
    # wt[p, cs, k, d] = conv_w[k, d, cs*128+p]
    wt = np.ascontiguousarray(
        conv_w.astype(BF16).reshape(KERNEL, DK, CS, P).transpose(3, 2, 0, 1)
    ).reshape(P, CS * KERNEL * DK)
    mask = np.zeros((P, 64), BF16)
    for p in range(P):
        a = p // 16
        mask[p, a * KERNEL : (a + 1) * KERNEL] = 1.0

    in_maps = []
    for c in range(NCORES):
        n0 = c * NODES_PC
        idxc = idx[n0 : n0 + NODES_PC].reshape(-1).astype(np.int16)
        wrapped = np.ascontiguousarray(idxc.reshape(E_PC // 16, 16).T)  # [16, 576]
        idx16 = np.ascontiguousarray(np.tile(wrapped, (8, 1)))  # [128, 576]
        own = np.repeat(ctr[n0 : n0 + NODES_PC], K_NB, axis=0)  # [9216, 2]
        ctro = np.ascontiguousarray(
            own.reshape(NG, P, 2).transpose(1, 0, 2)
        ).reshape(P, 2 * NG)
        nbr = ctr[idx[n0 : n0 + NODES_PC].reshape(-1)]  # [9216, 2]
        ctrn = np.ascontiguousarray(
            nbr.reshape(NG, P, 2).transpose(1, 0, 2)
        ).reshape(P, 2 * NG)
        gwc = gwf[n0 : n0 + NODES_PC].reshape(-1)
        gwp = np.ascontiguousarray(gwc.reshape(NG, P).T)  # [128, 72]
        in_maps.append(
            {
                "feat": feat_tbl,
                "idx16": idx16,
                "ctro": ctro,
                "ctrn": ctrn,
                "gw": gwp,
                "wt": wt,
                "mask": mask,
                "kpar": kpar,
            }
        )
    return in_maps, scalars


def _run(inputs, trace=False):
    global LAST_EXEC_NS
    in_maps, scalars = _host_pack(inputs)
    nc = _build_program(scalars)
    res = bass_utils.run_bass_kernel_spmd(
        nc, in_maps, core_ids=list(range(NCORES)), trace=trace
    )
    LAST_EXEC_NS = res.exec_time_ns
    out = np.concatenate(
        [np.asarray(res.results[c]["out"], np.float32) for c in range(NCORES)],
        axis=0,
    )
    return out.reshape(B, N, OUT), res


def kernel(**inputs) -> np.ndarray:
    out, _ = _run(inputs, trace=False)
    return out


# revision 18
# speedup vs baseline: 1.0975x; 1.0975x over previous
"""CgsGraphConv (gnn message passing) Trainium2 kernel.

Data-parallel over nodes: each of the 8 cores owns BN/8 = 576 nodes
(9216 edges).  Neighbor indices are global, so every core carries the
full (bf16) node-feature table in its own HBM and gathers neighbor rows
with dma_gather.

Per-core pipeline:
  1. dma_gather centre rows (bit-packed fp32 in a 256B-padded table),
     compute per-edge polar coords + Gaussian mixture edge weights
     (ACT/DVE), normalize, multiply by graph weights.
  2. dma_gather neighbor feature rows (bf16, 2KB each) in 9 chunks.
  3. Aggregation matmuls: per group of 128 edges (8 nodes), lhsT =
     gathered features [128e x 128c], rhs = block-diag masked edge
     weights [128e x 64(node x kernel)] -> psum [128c x 64] = agg^T.
  4. Projection matmuls: contract c in 8 chunks of 128; lhsT = agg^T
     slice [128c x nodes], rhs = conv_w^T slice [128c x 128d] ->
     psum [nodes x 128d]; ReLU -> out rows.
"""

import os
import sys

for _p in ("/opt/trn_rl_repo",):
    if _p not in sys.path and os.path.isdir(_p):
        sys.path.insert(0, _p)

from contextlib import ExitStack

import numpy as np
import ml_dtypes

import concourse.bass as bass
from concourse import bacc
import concourse.mybir as mybir
import concourse.tile as tile
from concourse import bass_utils

BF16 = ml_dtypes.bfloat16

B, N, C = 128, 36, 1024
K_NB = 16
KERNEL = 8
OUT = 1024
DK = OUT // KERNEL          # 128
BN = B * N                  # 4608
NCORES = 8
NODES_PC = BN // NCORES     # 576
E_PC = NODES_PC * K_NB      # 9216
P = 128
NG = E_PC // P              # 72 groups of 128 edges (8 nodes each)
NCALLS = 9                  # feature gather calls, 1024 idx each
GPC = NG // NCALLS          # 8 groups per gather call
CS = C // P                 # 8 contraction chunks
PI = float(np.pi)
HALF_PI = PI / 2.0

LAST_EXEC_NS = None


def _build_program(scalars):
    """Build the SPMD Bass program (identical on all cores)."""
    f32 = mybir.dt.float32
    bf16 = mybir.dt.bfloat16
    i16 = mybir.dt.int16
    u16 = mybir.dt.uint16
    AF = mybir.ActivationFunctionType

    nc = bacc.Bacc("TRN2", target_bir_lowering=False, debug=False,
                   num_devices=NCORES, num_swdge_queues=2)

    feat = nc.dram_tensor("feat", [BN, C], bf16, kind="ExternalInput").ap()
    idxd = nc.dram_tensor("idx16", [P, E_PC // 16], i16, kind="ExternalInput").ap()
    ctro = nc.dram_tensor("ctro", [P, 2 * NG], f32, kind="ExternalInput").ap()
    ctrn = nc.dram_tensor("ctrn", [P, 2 * NG], f32, kind="ExternalInput").ap()
    gwd = nc.dram_tensor("gw", [P, NG], f32, kind="ExternalInput").ap()
    wtd = nc.dram_tensor("wt", [P, CS * KERNEL * DK], bf16, kind="ExternalInput").ap()
    maskd = nc.dram_tensor("mask", [P, 64], bf16, kind="ExternalInput").ap()
    kpard = nc.dram_tensor("kpar", [P, 4 * KERNEL], f32, kind="ExternalInput").ap()
    outd = nc.dram_tensor("out", [NODES_PC, OUT], f32, kind="ExternalOutput").ap()

    with tile.TileContext(nc) as tc, ExitStack() as ctx:
        constp = ctx.enter_context(tc.tile_pool(name="const", bufs=1))
        wmp = ctx.enter_context(tc.tile_pool(name="wmath", bufs=1))

        idx_sb = constp.tile([P, E_PC // 16], i16)
        nc.sync.dma_start(idx_sb[:], idxd)
        # issue all feature gathers as early as possible (GpSimd is the
        # serial bottleneck; everything else overlaps them)
        fgp = ctx.enter_context(tc.tile_pool(name="fg", bufs=3))
        nidx_reg = nc.gpsimd.to_reg(GPC * P)
        fgs = []
        for j in range(NCALLS):
            fg = fgp.tile([P, GPC * C], bf16, name=f"fg{j}", tag="fg")
            fgv = fg[:].rearrange("p (s e) -> p s e", e=C)
            nc.gpsimd.dma_gather(
                out_ap=fgv,
                in_ap=feat,
                idxs_ap=idx_sb[:, 64 * j : 64 * (j + 1)],
                num_idxs=GPC * P,
                num_idxs_reg=nidx_reg,
                elem_size=C,
                queue_num=j % 2,
            )
            fgs.append(fgv)
        ctr_own = constp.tile([P, 2 * NG], f32)
        nc.sync.dma_start(ctr_own[:], ctro)
        ctr_nbr = constp.tile([P, 2 * NG], f32)
        nc.sync.dma_start(ctr_nbr[:], ctrn)
        gw_sb = constp.tile([P, NG], f32)
        nc.sync.dma_start(gw_sb[:], gwd)
        mask_sb = constp.tile([P, 64], bf16)
        nc.sync.dma_start(mask_sb[:], maskd)
        wt_sb = constp.tile([P, CS * KERNEL * DK], bf16)
        nc.sync.dma_start(wt_sb[:], wtd)

        cj = ctr_nbr[:]  # [P, 2*NG]: (g, (x, y)) of neighbor (host-packed)

        # --- per-edge polar coords ------------------------------------
        dxy = wmp.tile([P, 2 * NG], f32)
        sqt = wmp.tile([P, 2 * NG], f32)
        rho2 = wmp.tile([P, NG], f32)
        rho = wmp.tile([P, NG], f32)
        den = wmp.tile([P, NG], f32)
        rec = wmp.tile([P, NG], f32)
        rat = wmp.tile([P, NG], f32)
        aa = wmp.tile([P, NG], f32)

        nc.vector.tensor_sub(dxy[:], ctr_own[:], cj)
        nc.vector.tensor_mul(sqt[:], dxy[:], dxy[:])
        sq2 = sqt[:].rearrange("p (g two) -> p g two", two=2)
        d2 = dxy[:].rearrange("p (g two) -> p g two", two=2)
        nc.vector.tensor_add(rho2[:], sq2[:, :, 0], sq2[:, :, 1])
        # rho = sqrt(rho2) via exp(0.5*ln(rho2)): keeps ACT inside the
        # natural_log_exp table set (sqrt lives in a different set and
        # would cost an extra ~2.7us table load on the critical path)
        nc.vector.tensor_scalar_max(rho2[:], rho2[:], 1e-30)
        nc.scalar.activation(rho[:], rho2[:], AF.Ln)
        nc.scalar.activation(rho[:], rho[:], AF.Exp, scale=0.5)
        # theta = 2*atan(dx / (rho + dy)); guard rho+dy==0 (self loops)
        nc.vector.tensor_add(den[:], rho[:], d2[:, :, 1])
        nc.vector.tensor_scalar_max(den[:], den[:], 1e-20)
        nc.vector.reciprocal(rec[:], den[:])
        nc.vector.tensor_mul(rat[:], d2[:, :, 0], rec[:])
        # HW Arctan domain is [-pi/2, pi/2]; range-reduce:
        #   atan(t) = sign(t) * ( f*(pi/2) + (1-2f)*atan(min(|t|, 1/|t|)) ),
        #   f = (|t| > 1)
        abt = wmp.tile([P, NG], f32)
        invt = wmp.tile([P, NG], f32)
        mint = wmp.tile([P, NG], f32)
        att = wmp.tile([P, NG], f32)
        fgt = wmp.tile([P, NG], f32)
        qt = wmp.tile([P, NG], f32)
        sgn = wmp.tile([P, NG], f32)
        nc.scalar.activation(abt[:], rat[:], AF.Abs)
        nc.vector.tensor_scalar_max(abt[:], abt[:], 1e-30)
        nc.vector.reciprocal(invt[:], abt[:])
        nc.vector.tensor_tensor(mint[:], abt[:], invt[:], op=mybir.AluOpType.min)
        nc.scalar.activation(att[:], mint[:], AF.Arctan)
        nc.vector.tensor_scalar(
            fgt[:], abt[:], 1.0, None, op0=mybir.AluOpType.is_gt
        )
        # q = 1 - 2f ; aa_abs = q*at + f*pi/2 ; aa = sign(rat) * aa_abs
        # (two-scalar tensor_scalar hits a DVE slow path -- use single ops)
        nc.vector.tensor_scalar_mul(qt[:], fgt[:], -2.0)
        nc.vector.tensor_scalar_add(qt[:], qt[:], 1.0)
        nc.vector.tensor_mul(att[:], att[:], qt[:])
        nc.vector.tensor_scalar_mul(fgt[:], fgt[:], HALF_PI)
        nc.vector.tensor_add(att[:], att[:], fgt[:])
        nc.scalar.activation(sgn[:], rat[:], AF.Sign)
        nc.vector.tensor_mul(aa[:], sgn[:], att[:])  # = theta / 2

        # --- Gaussian mixture weights over the 8 kernels --------------
        # k-batched on [P, NG, KERNEL] tiles with per-kernel parameter
        # rows (kpar input) broadcast along g; per-partition float biases
        # for ACT ops are constant across k.
        _bias_cache = {}

        def bias_ap(val):
            val = float(val)
            if val == 0.0:
                return 0.0  # pre-registered const
            if val not in _bias_cache:
                t = wmp.tile([P, 1], f32, name=f"bias{len(_bias_cache)}")
                nc.vector.memset(t[:], val)
                _bias_cache[val] = t
            return _bias_cache[val][:]

        AL = mybir.AluOpType
        kpar_sb = constp.tile([P, 4 * KERNEL], f32)
        nc.sync.dma_start(kpar_sb[:], kpard)
        kview = kpar_sb[:].rearrange("p (f k) -> p f k", k=KERNEL)

        def kp(i):  # [P, NG, KERNEL] broadcast of param row i
            return kview[:, i : i + 1, :].broadcast_to([P, NG, KERNEL])

        def gb(t):  # [P, NG] tile broadcast along k
            return t[:].unsqueeze(2).broadcast_to([P, NG, KERNEL])

        dkt = wmp.tile([P, NG * KERNEL], f32)
        ukt = wmp.tile([P, NG * KERNEL], f32)
        a1t = wmp.tile([P, NG * KERNEL], f32)
        vvt = wmp.tile([P, NG * KERNEL], f32)
        wall = wmp.tile([P, NG * KERNEL], f32)
        d3 = dkt[:].rearrange("p (g k) -> p g k", k=KERNEL)
        u3 = ukt[:].rearrange("p (g k) -> p g k", k=KERNEL)
        a3 = a1t[:].rearrange("p (g k) -> p g k", k=KERNEL)
        v3 = vvt[:].rearrange("p (g k) -> p g k", k=KERNEL)
        wall3 = wall[:].rearrange("p (g k) -> p g k", k=KERNEL)
        # u = (rho - mr)^2 * (0.5/var_r)
        nc.vector.tensor_tensor(d3, gb(rho), kp(0), op=AL.subtract)
        nc.scalar.activation(ukt[:], dkt[:], AF.Square)
        nc.vector.tensor_tensor(u3, u3, kp(1), op=AL.mult)
        # circular distance via half angle: m/2 = pi/2 - ||aa - mt/2| - pi/2|
        nc.vector.tensor_tensor(d3, gb(aa), kp(2), op=AL.subtract)
        nc.scalar.activation(a1t[:], dkt[:], AF.Abs)
        nc.scalar.activation(a1t[:], a1t[:], AF.Abs, bias=bias_ap(-HALF_PI))
        # v = (pi/2 - a1)^2 * (2/var_t)
        nc.scalar.activation(vvt[:], a1t[:], AF.Square, bias=bias_ap(HALF_PI),
                             scale=-1.0)
        nc.vector.tensor_tensor(v3, v3, kp(3), op=AL.mult)
        nc.vector.tensor_add(ukt[:], ukt[:], vvt[:])
        nc.scalar.activation(wall[:], ukt[:], AF.Exp, scale=-1.0)

        # --- normalize + graph weights + block-diag masks -------------
        wsum = wmp.tile([P, NG], f32)
        wrec = wmp.tile([P, NG], f32)
        gwn = wmp.tile([P, NG], f32)
        nc.vector.tensor_add(wsum[:], wall3[:, :, 0], wall3[:, :, 1])
        for k in range(2, KERNEL):
            nc.vector.tensor_add(wsum[:], wsum[:], wall3[:, :, k])
        nc.vector.reciprocal(wrec[:], wsum[:])
        nc.vector.tensor_mul(gwn[:], gw_sb[:], wrec[:])

        wnorm = wmp.tile([P, NG * KERNEL], bf16)
        nc.vector.tensor_tensor(
            wnorm[:].rearrange("p (g k) -> p g k", k=KERNEL),
            wall3,
            gwn[:].unsqueeze(2).broadcast_to([P, NG, KERNEL]),
            op=mybir.AluOpType.mult,
        )
        # bd[p, g, a, k] = wnorm[p, g, k] * mask[p, a, k]
        bdall = wmp.tile([P, NG * 64], bf16)
        nc.vector.tensor_tensor(
            bdall[:].rearrange("p (g a k) -> p g a k", a=8, k=KERNEL),
            wnorm[:]
            .rearrange("p (g k) -> p g k", k=KERNEL)
            .unsqueeze(2)
            .broadcast_to([P, NG, 8, KERNEL]),
            mask_sb[:]
            .rearrange("p (a k) -> p a k", k=KERNEL)
            .unsqueeze(1)
            .broadcast_to([P, NG, 8, KERNEL]),
            op=mybir.AluOpType.mult,
        )
        bd3 = bdall[:].rearrange("p (g m) -> p g m", m=64)

        # --- aggregation matmuls --------------------------------------
        aggp = ctx.enter_context(tc.tile_pool(name="aggT", bufs=1))
        outp = ctx.enter_context(tc.tile_pool(name="outb", bufs=2))
        aggpsp = ctx.enter_context(tc.tile_pool(name="aps", bufs=3, space="PSUM"))
        projpsp = ctx.enter_context(tc.tile_pool(name="pps", bufs=4, space="PSUM"))

        aggT = aggp.tile([P, CS * NODES_PC * KERNEL], bf16)
        aggT3 = aggT[:].rearrange("p (cs ik) -> p cs ik", cs=CS)
        aggT4 = aggT[:].rearrange(
            "p (cs i k) -> p cs i k", cs=CS, k=KERNEL
        )  # [P, CS, NODES_PC, KERNEL]
        wtv = wt_sb[:].rearrange("p (cs k d) -> p cs k d", cs=CS, k=KERNEL)
        iblocks = [(0, 128), (128, 128), (256, 128), (384, 128), (512, 64)]

        proj_q = [(b, k) for b in range(4) for k in range(KERNEL)]

        outbs = {}

        def emit_proj_slice(ib, k):
            i0, mi = iblocks[ib]
            if ib not in outbs:
                outbs[ib] = outp.tile([P, OUT], mybir.dt.float32,
                                      name=f"outb{ib}", tag="outb")
            outb = outbs[ib]
            pps = projpsp.tile([P, DK], mybir.dt.float32,
                               name=f"pps{ib}_{k}", tag="pps")
            for cs in range(CS):
                nc.tensor.matmul(
                    pps[:mi, :],
                    aggT4[:, cs, i0 : i0 + mi, k],
                    wtv[:, cs, k, :],
                    start=(cs == 0),
                    stop=(cs == CS - 1),
                )
            nc.scalar.activation(
                outb[:mi, k * DK : (k + 1) * DK], pps[:mi, :], AF.Relu
            )
            if k == KERNEL - 1:
                nc.sync.dma_start(outd[i0 : i0 + mi, :], outb[:mi, :])

        for j in range(NCALLS):
            fgv = fgs[j]
            if j == NCALLS - 1:
                # last call: drain every already-eligible projection slice
                # before its aggregation so PE works while the final
                # gather is still in flight
                while proj_q and proj_q[0][0] <= (j - 2) // 2:
                    emit_proj_slice(*proj_q.pop(0))
            for s in range(GPC):
                g = GPC * j + s
                aps = aggpsp.tile([P, CS * 64], mybir.dt.float32,
                                  name=f"aps{g}", tag="aps")
                for cs in range(CS):
                    nc.tensor.matmul(
                        aps[:, cs * 64 : (cs + 1) * 64],
                        fgv[:, s, cs * P : (cs + 1) * P],
                        bd3[:, g, :],
                        start=True,
                        stop=True,
                    )
                if g % 2 == 0:
                    nc.vector.tensor_copy(
                        aggT3[:, :, g * 64 : (g + 1) * 64],
                        aps[:].rearrange("p (cs m) -> p cs m", cs=CS),
                    )
                else:
                    nc.scalar.copy(
                        aggT3[:, :, g * 64 : (g + 1) * 64],
                        aps[:].rearrange("p (cs m) -> p cs m", cs=CS),
                    )
                # interleave one projection slice (8 matmuls) per group,
                # once its i-block's aggregates are fully written:
                # ib b needs gather calls 2b, 2b+1 -> ready from call 2b+2
                if proj_q and proj_q[0][0] <= j // 2 - 1:
                    emit_proj_slice(*proj_q.pop(0))
        while proj_q:
            emit_proj_slice(*proj_q.pop(0))
        for k in range(KERNEL):
            emit_proj_slice(4, k)

    nc.compile()
    return nc


def _host_pack(inputs):
    """Shard + pack host inputs into per-core in_maps and scalars."""
    nf = np.ascontiguousarray(
        np.asarray(inputs["node_feats"], np.float32).reshape(BN, C)
    )
    ctr = np.ascontiguousarray(
        np.asarray(inputs["node_centre"], np.float32).reshape(BN, 2)
    )
    idx = np.asarray(inputs["neighbor_idx"]).reshape(BN, K_NB)
    gwf = np.asarray(inputs["graph_weights"], np.float32).reshape(BN, K_NB)
    mean_rho = np.asarray(inputs["mean_rho"], np.float32).reshape(KERNEL)
    mean_theta = np.asarray(inputs["mean_theta"], np.float32).reshape(KERNEL)
    prec_rho = np.asarray(inputs["precision_rho"], np.float32).reshape(KERNEL)
    prec_theta = np.asarray(inputs["precision_theta"], np.float32).reshape(KERNEL)
    conv_w = np.asarray(inputs["conv_w"], np.float32)

    scalars = {}
    kpar1 = np.concatenate([
        mean_rho,
        0.5 / (1e-14 + prec_rho ** 2),
        mean_theta / 2.0,
        2.0 / (1e-14 + prec_theta ** 2),
    ]).astype(np.float32)  # [32]
    kpar = np.ascontiguousarray(np.tile(kpar1[None, :], (P, 1)))

    feat_tbl = nf.astype(BF16)
    # wt[p, cs, k, d] = conv_w[k, d, cs*128+p]
    wt = np.ascontiguousarray(
        conv_w.astype(BF16).reshape(KERNEL, DK, CS, P).transpose(3, 2, 0, 1)
    ).reshape(P, CS * KERNEL * DK)
    mask = np.zeros((P, 64), BF16)
    for p in range(P):
        a = p // 16
        mask[p, a * KERNEL : (a + 1) * KERNEL] = 1.0

    in_maps = []
    for c in range(NCORES):
        n0 = c * NODES_PC
        idxc = idx[n0 : n0 + NODES_PC].reshape(-1).astype(np.int16)
        wrapped = np.ascontiguousarray(idxc.reshape(E_PC // 16, 16).T)  # [16, 576]
        idx16 = np.ascontiguousarray(np.tile(wrapped, (8, 1)))  # [128, 576]
        own = np.repeat(ctr[n0 : n0 + NODES_PC], K_NB, axis=0)  # [9216, 2]
        ctro = np.ascontiguousarray(
            own.reshape(NG, P, 2).transpose(1, 0, 2)
        ).reshape(P, 2 * NG)
        nbr = ctr[idx[n0 : n0 + NODES_PC].reshape(-1)]  # [9216, 2]
        ctrn = np.ascontiguousarray(
            nbr.reshape(NG, P, 2).transpose(1, 0, 2)
        ).reshape(P, 2 * NG)
        gwc = gwf[n0 : n0 + NODES_PC].reshape(-1)
        gwp = np.ascontiguousarray(gwc.reshape(NG, P).T)  # [128, 72]
        in_maps.append(
            {
                "feat": feat_tbl,
                "idx16": idx16,
                "ctro": ctro,
                "ctrn": ctrn,
                "gw": gwp,
                "wt": wt,
                "mask": mask,
                "kpar": kpar,
            }
        )
    return in_maps, scalars


def _run(inputs, trace=False):
    global LAST_EXEC_NS
    in_maps, scalars = _host_pack(inputs)
    nc = _build_program(scalars)
    res = bass_utils.run_bass_kernel_spmd(
        nc, in_maps, core_ids=list(range(NCORES)), trace=trace
    )
    LAST_EXEC_NS = res.exec_time_ns
    out = np.concatenate(
        [np.asarray(res.results[c]["out"], np.float32) for c in range(NCORES)],
        axis=0,
    )
    return out.reshape(B, N, OUT), res


def kernel(**inputs) -> np.ndarray:
    out, _ = _run(inputs, trace=False)
    return out
